# revision 16
# baseline (speedup 1.0000x reference)
"""BoT tokenizer kernel for Trainium2 (Bass/Tile), 8-core data parallel.

All 25 output tokens are computed on the TensorEngine as bf16 matmuls with
an exact fp32 -> 3x bf16 mantissa split (8+8+8 = 24 bits):

    x = a0 + a1 + a2 (each bf16, split exact by construction)
    x*w = sum_{i,j} ai*wj   (each bf16 product is exact in fp32)

 - single-feature token k: K=12 matmul (9 cross products + 3 bias rows
   against a ones column)
 - fore token: 9 features -> K = 9*9+3 = 84
 - palm token: 7 features -> K = 7*9+3 = 66

bf16 matmuls stream 1 col/cycle (vs 4 for fp32), so the PE produces each
[128,512] token tile in ~215ns. PSUM->SBUF copies are split between
VectorE and ScalarE. The kernel is then purely output-DMA bound.

v2: the output is written to HBM as bf16 (down-converted during the
PSUM->SBUF copy) and up-cast to fp32 on the host. Compute stays exact;
the only error is the final bf16 rounding (~1e-3 normalized l2), well
inside the 2e-2 gate. This halves the dominant HBM write traffic:
each core writes 1024*25*512*2 = 26.2 MB. Singles inputs are stored
compact in HBM (no 32-row zero padding) and DMA'd into the padded
SBUF layout, cutting input DMA from 2.9 MB to 1.3 MB per core.
"""

import numpy as np

FORE_IDX = [0, 1, 2, 27, 28, 32, 33, 34, 38]
PALM_IDX = [4, 29, 30, 31, 35, 36, 37]
SINGLE_IDX = [3] + list(range(5, 27))

B = 8192
D = 512
T = 25
N_CORES = 8
B_LOC = B // N_CORES          # 1024 rows per core
CHUNK = 128
N_CHUNKS = B_LOC // CHUNK     # 8
ROW = T * D                   # 12800
NS = 23

# token id for single sensor k: k=0 -> token 1 (wrist), k>=1 -> token k+2
TOK_OF_SINGLE = [1] + list(range(3, 25))
# out-tile token groups for finer DMA pipelining
GROUPS = [(0, 6), (6, 12), (12, 19), (19, 25)]
KF = 9 * 9 + 3                # 84
KP = 7 * 9 + 3                # 66
KS = 12
# singles packed 3 per tile at 32-partition offsets (matmul base partition
# must be 0/32/64)
S_TILES = [(a, min(a + 3, NS)) for a in range(0, NS, 3)]
S_STRIDE = 32

_prog_cache = {}


def _k_of_tok(t):
    return 0 if t == 1 else t - 2


def _build_program():
    import concourse.bacc as bacc
    import concourse.mybir as mybir
    import concourse.tile as tile
    from concourse.bass import ts

    f32 = mybir.dt.float32
    bf16 = mybir.dt.bfloat16
    nc = bacc.Bacc("TRN2", target_bir_lowering=False, debug=False,
                   num_devices=N_CORES)

    lf_d = nc.dram_tensor("lf", [KF, B_LOC], bf16, kind="ExternalInput")
    lp_d = nc.dram_tensor("lp", [KP, B_LOC], bf16, kind="ExternalInput")
    rf_d = nc.dram_tensor("rf", [KF, D], bf16, kind="ExternalInput")
    rp_d = nc.dram_tensor("rp", [KP, D], bf16, kind="ExternalInput")
    ls_d = [nc.dram_tensor(f"ls{i}", [(b - a - 1) * S_STRIDE + KS, B_LOC],
                           bf16, kind="ExternalInput")
            for i, (a, b) in enumerate(S_TILES)]
    rs_d = [nc.dram_tensor(f"rs{i}", [(b - a - 1) * S_STRIDE + KS, D],
                           bf16, kind="ExternalInput")
            for i, (a, b) in enumerate(S_TILES)]
    out_d = nc.dram_tensor("out", [B_LOC, ROW], bf16, kind="ExternalOutput")

    with tile.TileContext(nc) as tc:
        with (
            tc.tile_pool(name="cst", bufs=1) as cst,
            tc.tile_pool(name="op", bufs=3) as op,
            tc.tile_pool(name="pp", bufs=4, space="PSUM") as pp,
        ):
            # Each dma_start costs ~700ns on its issuing sequencer, so use
            # one per tensor and keep vector/scalar free for PSUM copies:
            # gpsimd issues all lhsT loads, sync all rhs loads. The singles
            # are compact in HBM and land in strided 32-partition SBUF
            # slots via a rearranged AP in a single DMA each.
            lf_s = cst.tile([KF, B_LOC], bf16)
            nc.gpsimd.dma_start(out=lf_s[:], in_=lf_d[:])
            rf_s = cst.tile([KF, D], bf16)
            nc.sync.dma_start(out=rf_s[:], in_=rf_d[:])
            ls_s, rs_s = [], []
            for i, (a, b) in enumerate(S_TILES):
                rows = (b - a - 1) * S_STRIDE + KS
                lt = cst.tile([rows, B_LOC], bf16, name=f"ls{i}_s")
                nc.gpsimd.dma_start(out=lt[:], in_=ls_d[i][:])
                ls_s.append(lt)
                rt = cst.tile([rows, D], bf16, name=f"rs{i}_s")
                nc.sync.dma_start(out=rt[:], in_=rs_d[i][:])
                rs_s.append(rt)
                if i == 0:
                    # palm operands next: token 2 is needed right after
                    # fore (t0) and wrist (t1, sensor tile 0)
                    lp_s = cst.tile([KP, B_LOC], bf16)
                    nc.gpsimd.dma_start(out=lp_s[:], in_=lp_d[:])
                    rp_s = cst.tile([KP, D], bf16)
                    nc.sync.dma_start(out=rp_s[:], in_=rp_d[:])

            def operands(t, c):
                if t == 0:
                    return lf_s[:, ts(c, CHUNK)], rf_s[:]
                if t == 2:
                    return lp_s[:, ts(c, CHUNK)], rp_s[:]
                k = _k_of_tok(t)
                i = k // 3
                off = S_STRIDE * (k - S_TILES[i][0])
                return (ls_s[i][off:off + KS, ts(c, CHUNK)],
                        rs_s[i][off:off + KS, :])

            for c in range(N_CHUNKS):
                ncopy = 0
                for gi, (t0, t1) in enumerate(GROUPS):
                    o_t = op.tile([CHUNK, (t1 - t0) * D], bf16,
                                  tag=f"out{gi}", bufs=3)
                    # 2 matmuls -> one 2-bank PSUM tile. Measured per-token
                    # copy costs: scalar does pairs best (604ns/tok), vector
                    # does singles best (661ns/tok); alternating units gives
                    # scalar 13 tokens and vector 12 -> ~7.9us/chunk each.
                    t = t0
                    while t < t1:
                        te = min(t + 2, t1)
                        w = (te - t) * D
                        p_t = pp.tile([CHUNK, 2 * D], f32)
                        for tt in range(t, te):
                            lhsT, rhs = operands(tt, c)
                            nc.tensor.matmul(p_t[:, ts(tt - t, D)], lhsT,
                                             rhs, start=True, stop=True)
                        dst = o_t[:, (t - t0) * D:(te - t0) * D]
                        if ncopy % 2 == 0:
                            nc.scalar.copy(dst, p_t[:, 0:w])
                        else:
                            for j in range(te - t):
                                nc.vector.tensor_copy(
                                    o_t[:, (t - t0 + j) * D:(t - t0 + j + 1) * D],
                                    p_t[:, ts(j, D)])
                        if c < 2:
                            # ramp: per-unit DMA keeps the queues fed while
                            # production is still slow (PE p-state, input
                            # arrival); losing queue time early is never
                            # recovered since steady-state headroom is ~3%
                            dma_eng = nc.sync if ncopy % 2 == 0 else nc.gpsimd
                            dma_eng.dma_start(
                                out=out_d[ts(c, CHUNK), t * D:te * D],
                                in_=o_t[:, (t - t0) * D:(te - t0) * D])
                        ncopy += 1
                        t = te
                    if c >= 2:
                        dma_eng = nc.sync if gi % 2 == 0 else nc.gpsimd
                        dma_eng.dma_start(
                            out=out_d[ts(c, CHUNK), t0 * D:t1 * D], in_=o_t[:])

    nc.compile()
    return nc


def _split3(v):
    """Exact fp32 -> (bf16, bf16, bf16) mantissa split: v = s0+s1+s2."""
    import ml_dtypes
    bf = ml_dtypes.bfloat16
    v = np.asarray(v, np.float32)
    s0 = v.astype(bf)
    r1 = v - s0.astype(np.float32)
    s1 = r1.astype(bf)
    r2 = r1 - s1.astype(np.float32)
    s2 = r2.astype(bf)
    return s0, s1, s2


def _lhs_rows(xcols):
    """lhsT rows for a feature block: a0,a0,a0,a1,a1,a1,a2,a2,a2 per feat.

    xcols: [B, F] fp32 -> [9F, B] bf16"""
    import ml_dtypes
    Bn, F = xcols.shape
    s0, s1, s2 = _split3(xcols)          # each [B, F]
    out = np.empty((F, 9, Bn), dtype=ml_dtypes.bfloat16)
    for i, s in enumerate((s0, s1, s2)):
        out[:, 3 * i:3 * i + 3, :] = s.T[:, None, :]
    return out.reshape(9 * F, Bn)


def _rhs_rows(wcols):
    """rhs rows for a feature block: w0,w1,w2,w0,w1,w2,w0,w1,w2 per feat.

    wcols: [F, D] fp32 -> [9F, D] bf16"""
    import ml_dtypes
    F, Dn = wcols.shape
    s0, s1, s2 = _split3(wcols)
    out = np.empty((F, 3, 3, Dn), dtype=ml_dtypes.bfloat16)
    for j, s in enumerate((s0, s1, s2)):
        out[:, :, j, :] = s[:, None, :]
    return out.reshape(9 * F, Dn)


def _host_prep(x, Wf, bf_, Wp, bp, Ws, bs):
    import ml_dtypes
    bf16 = ml_dtypes.bfloat16

    ones3 = np.ones((3, B), dtype=bf16)

    def bias_rows(bias):
        b0, b1, b2 = _split3(bias)       # [D] each
        return np.stack([b0, b1, b2])    # [3, D]

    # fore: lhsT [84, B], rhs [84, D]
    lf = np.concatenate([_lhs_rows(x[:, FORE_IDX]), ones3])
    rf = np.concatenate([_rhs_rows(np.asarray(Wf.T)), bias_rows(bf_)])
    # palm: [66, *]
    lp = np.concatenate([_lhs_rows(x[:, PALM_IDX]), ones3])
    rp = np.concatenate([_rhs_rows(np.asarray(Wp.T)), bias_rows(bp)])

    # singles: per sensor a [12, *] block at 32-partition offsets (pad rows
    # zero) so each SBUF tile loads with one plain-AP DMA
    ls_all = np.zeros((NS * S_STRIDE, B), dtype=bf16)
    rs_all = np.zeros((NS * S_STRIDE, D), dtype=bf16)
    xs = x[:, SINGLE_IDX]                # [B, 23]
    for k in range(NS):
        o = S_STRIDE * k
        ls_all[o:o + 9] = _lhs_rows(xs[:, k:k + 1])
        ls_all[o + 9:o + KS] = ones3
        rs_all[o:o + 9] = _rhs_rows(Ws[k:k + 1])
        rs_all[o + 9:o + KS] = bias_rows(bs[k])
    return lf, rf, lp, rp, ls_all, rs_all


def kernel(x, Wf, bf, Wp, bp, Ws, bs, _trace=False, _spmd_kwargs=None):
    from concourse.bass_utils import run_bass_kernel_spmd

    x = np.asarray(x, np.float32)
    lf, rf, lp, rp, ls_all, rs_all = _host_prep(
        x, np.asarray(Wf, np.float32), np.asarray(bf, np.float32),
        np.asarray(Wp, np.float32), np.asarray(bp, np.float32),
        np.asarray(Ws, np.float32), np.asarray(bs, np.float32))

    if "nc" not in _prog_cache:
        _prog_cache["nc"] = _build_program()
    nc = _prog_cache["nc"]

    in_maps = []
    for i in range(N_CORES):
        sl = slice(i * B_LOC, (i + 1) * B_LOC)
        m = {
            "lf": np.ascontiguousarray(lf[:, sl]),
            "lp": np.ascontiguousarray(lp[:, sl]),
            "rf": rf,
            "rp": rp,
        }
        for j, (a, b) in enumerate(S_TILES):
            rows = (b - a - 1) * S_STRIDE + KS
            o = S_STRIDE * a
            m[f"ls{j}"] = np.ascontiguousarray(ls_all[o:o + rows, sl])
            m[f"rs{j}"] = np.ascontiguousarray(rs_all[o:o + rows])
        in_maps.append(m)

    kwargs = dict(_spmd_kwargs or {})
    res = run_bass_kernel_spmd(nc, in_maps, core_ids=list(range(N_CORES)),
                               trace=_trace, **kwargs)
    out = np.concatenate(
        [np.asarray(r["out"]).astype(np.float32) for r in res.results], axis=0)
    if _trace:
        kernel.last_results = res
    return out.reshape(B, T, D)



# revision 17
# speedup vs baseline: 1.0273x; 1.0273x over previous
"""BoT tokenizer kernel for Trainium2 (Bass/Tile), 8-core data parallel.

All 25 output tokens are computed on the TensorEngine as bf16 matmuls with
an exact fp32 -> 3x bf16 mantissa split (8+8+8 = 24 bits):

    x = a0 + a1 + a2 (each bf16, split exact by construction)
    x*w = sum_{i,j} ai*wj   (each bf16 product is exact in fp32)

 - single-feature token k: K=12 matmul (9 cross products + 3 bias rows
   against a ones column)
 - fore token: 9 features -> K = 9*9+3 = 84
 - palm token: 7 features -> K = 7*9+3 = 66

bf16 matmuls stream 1 col/cycle (vs 4 for fp32), so the PE produces each
[128,512] token tile in ~215ns. PSUM->SBUF copies are split between
VectorE and ScalarE. The kernel is then purely output-DMA bound.

v2: the output is written to HBM as bf16 (down-converted during the
PSUM->SBUF copy) and up-cast to fp32 on the host. Compute stays exact;
the only error is the final bf16 rounding (~1e-3 normalized l2), well
inside the 2e-2 gate. This halves the dominant HBM write traffic:
each core writes 1024*25*512*2 = 26.2 MB. Singles inputs are stored
compact in HBM (no 32-row zero padding) and DMA'd into the padded
SBUF layout, cutting input DMA from 2.9 MB to 1.3 MB per core.
"""

import numpy as np

FORE_IDX = [0, 1, 2, 27, 28, 32, 33, 34, 38]
PALM_IDX = [4, 29, 30, 31, 35, 36, 37]
SINGLE_IDX = [3] + list(range(5, 27))

B = 8192
D = 512
T = 25
N_CORES = 8
B_LOC = B // N_CORES          # 1024 rows per core
CHUNK = 128
N_CHUNKS = B_LOC // CHUNK     # 8
ROW = T * D                   # 12800
NS = 23

# token id for single sensor k: k=0 -> token 1 (wrist), k>=1 -> token k+2
TOK_OF_SINGLE = [1] + list(range(3, 25))
# out-tile token groups for finer DMA pipelining
GROUPS = [(0, 6), (6, 12), (12, 19), (19, 25)]
KF = 9 * 9 + 3                # 84
KP = 7 * 9 + 3                # 66
KS = 12
# singles packed 3 per tile at 32-partition offsets (matmul base partition
# must be 0/32/64)
S_TILES = [(a, min(a + 3, NS)) for a in range(0, NS, 3)]
S_STRIDE = 32

_prog_cache = {}


def _k_of_tok(t):
    return 0 if t == 1 else t - 2


def _build_program():
    import concourse.bacc as bacc
    import concourse.mybir as mybir
    import concourse.tile as tile
    from concourse.bass import ts

    f32 = mybir.dt.float32
    bf16 = mybir.dt.bfloat16
    nc = bacc.Bacc("TRN2", target_bir_lowering=False, debug=False,
                   num_devices=N_CORES)

    lf_d = nc.dram_tensor("lf", [KF, B_LOC], bf16, kind="ExternalInput")
    lp_d = nc.dram_tensor("lp", [KP, B_LOC], bf16, kind="ExternalInput")
    rf_d = nc.dram_tensor("rf", [KF, D], bf16, kind="ExternalInput")
    rp_d = nc.dram_tensor("rp", [KP, D], bf16, kind="ExternalInput")
    ls_d = [nc.dram_tensor(f"ls{i}", [(b - a - 1) * S_STRIDE + KS, B_LOC],
                           bf16, kind="ExternalInput")
            for i, (a, b) in enumerate(S_TILES)]
    rs_d = [nc.dram_tensor(f"rs{i}", [(b - a - 1) * S_STRIDE + KS, D],
                           bf16, kind="ExternalInput")
            for i, (a, b) in enumerate(S_TILES)]
    out_d = nc.dram_tensor("out", [B_LOC, ROW], bf16, kind="ExternalOutput")

    with tile.TileContext(nc) as tc:
        with (
            tc.tile_pool(name="cst", bufs=1) as cst,
            tc.tile_pool(name="op", bufs=3) as op,
            tc.tile_pool(name="pp", bufs=4, space="PSUM") as pp,
        ):
            # Each dma_start costs ~700ns on its issuing sequencer, so use
            # one per tensor and keep vector/scalar free for PSUM copies:
            # gpsimd issues all lhsT loads, sync all rhs loads. The singles
            # are compact in HBM and land in strided 32-partition SBUF
            # slots via a rearranged AP in a single DMA each.
            lf_s = cst.tile([KF, B_LOC], bf16)
            nc.gpsimd.dma_start(out=lf_s[:], in_=lf_d[:])
            rf_s = cst.tile([KF, D], bf16)
            nc.sync.dma_start(out=rf_s[:], in_=rf_d[:])
            ls_s, rs_s = [], []
            for i, (a, b) in enumerate(S_TILES):
                rows = (b - a - 1) * S_STRIDE + KS
                lt = cst.tile([rows, B_LOC], bf16, name=f"ls{i}_s")
                nc.gpsimd.dma_start(out=lt[:], in_=ls_d[i][:])
                ls_s.append(lt)
                rt = cst.tile([rows, D], bf16, name=f"rs{i}_s")
                nc.sync.dma_start(out=rt[:], in_=rs_d[i][:])
                rs_s.append(rt)
                if i == 0:
                    # palm operands next: token 2 is needed right after
                    # fore (t0) and wrist (t1, sensor tile 0)
                    lp_s = cst.tile([KP, B_LOC], bf16)
                    nc.gpsimd.dma_start(out=lp_s[:], in_=lp_d[:])
                    rp_s = cst.tile([KP, D], bf16)
                    nc.sync.dma_start(out=rp_s[:], in_=rp_d[:])

            def operands(t, c):
                if t == 0:
                    return lf_s[:, ts(c, CHUNK)], rf_s[:]
                if t == 2:
                    return lp_s[:, ts(c, CHUNK)], rp_s[:]
                k = _k_of_tok(t)
                i = k // 3
                off = S_STRIDE * (k - S_TILES[i][0])
                return (ls_s[i][off:off + KS, ts(c, CHUNK)],
                        rs_s[i][off:off + KS, :])

            for c in range(N_CHUNKS):
                ncopy = 0
                for gi, (t0, t1) in enumerate(GROUPS):
                    o_t = op.tile([CHUNK, (t1 - t0) * D], bf16,
                                  tag=f"out{gi}", bufs=3)
                    # 2 matmuls -> one 2-bank PSUM tile. Measured per-token
                    # copy costs: scalar does pairs best (604ns/tok), vector
                    # does singles best (661ns/tok); alternating units gives
                    # scalar 13 tokens and vector 12 -> ~7.9us/chunk each.
                    t = t0
                    while t < t1:
                        te = min(t + 2, t1)
                        w = (te - t) * D
                        p_t = pp.tile([CHUNK, 2 * D], f32)
                        for tt in range(t, te):
                            lhsT, rhs = operands(tt, c)
                            nc.tensor.matmul(p_t[:, ts(tt - t, D)], lhsT,
                                             rhs, start=True, stop=True)
                        dst = o_t[:, (t - t0) * D:(te - t0) * D]
                        if ncopy % 2 == 0:
                            nc.scalar.copy(dst, p_t[:, 0:w])
                        else:
                            for j in range(te - t):
                                nc.vector.tensor_copy(
                                    o_t[:, (t - t0 + j) * D:(t - t0 + j + 1) * D],
                                    p_t[:, ts(j, D)])
                        ncopy += 1
                        t = te
                    dma_eng = nc.sync if gi % 2 == 0 else nc.gpsimd
                    dma_eng.dma_start(
                        out=out_d[ts(c, CHUNK), t0 * D:t1 * D], in_=o_t[:])

    nc.compile()
    return nc


def _split3(v):
    """Exact fp32 -> (bf16, bf16, bf16) mantissa split: v = s0+s1+s2."""
    import ml_dtypes
    bf = ml_dtypes.bfloat16
    v = np.asarray(v, np.float32)
    s0 = v.astype(bf)
    r1 = v - s0.astype(np.float32)
    s1 = r1.astype(bf)
    r2 = r1 - s1.astype(np.float32)
    s2 = r2.astype(bf)
    return s0, s1, s2


def _lhs_rows(xcols):
    """lhsT rows for a feature block: a0,a0,a0,a1,a1,a1,a2,a2,a2 per feat.

    xcols: [B, F] fp32 -> [9F, B] bf16"""
    import ml_dtypes
    Bn, F = xcols.shape
    s0, s1, s2 = _split3(xcols)          # each [B, F]
    out = np.empty((F, 9, Bn), dtype=ml_dtypes.bfloat16)
    for i, s in enumerate((s0, s1, s2)):
        out[:, 3 * i:3 * i + 3, :] = s.T[:, None, :]
    return out.reshape(9 * F, Bn)


def _rhs_rows(wcols):
    """rhs rows for a feature block: w0,w1,w2,w0,w1,w2,w0,w1,w2 per feat.

    wcols: [F, D] fp32 -> [9F, D] bf16"""
    import ml_dtypes
    F, Dn = wcols.shape
    s0, s1, s2 = _split3(wcols)
    out = np.empty((F, 3, 3, Dn), dtype=ml_dtypes.bfloat16)
    for j, s in enumerate((s0, s1, s2)):
        out[:, :, j, :] = s[:, None, :]
    return out.reshape(9 * F, Dn)


def _host_prep(x, Wf, bf_, Wp, bp, Ws, bs):
    import ml_dtypes
    bf16 = ml_dtypes.bfloat16

    ones3 = np.ones((3, B), dtype=bf16)

    def bias_rows(bias):
        b0, b1, b2 = _split3(bias)       # [D] each
        return np.stack([b0, b1, b2])    # [3, D]

    # fore: lhsT [84, B], rhs [84, D]
    lf = np.concatenate([_lhs_rows(x[:, FORE_IDX]), ones3])
    rf = np.concatenate([_rhs_rows(np.asarray(Wf.T)), bias_rows(bf_)])
    # palm: [66, *]
    lp = np.concatenate([_lhs_rows(x[:, PALM_IDX]), ones3])
    rp = np.concatenate([_rhs_rows(np.asarray(Wp.T)), bias_rows(bp)])

    # singles: per sensor a [12, *] block at 32-partition offsets (pad rows
    # zero) so each SBUF tile loads with one plain-AP DMA
    ls_all = np.zeros((NS * S_STRIDE, B), dtype=bf16)
    rs_all = np.zeros((NS * S_STRIDE, D), dtype=bf16)
    xs = x[:, SINGLE_IDX]                # [B, 23]
    for k in range(NS):
        o = S_STRIDE * k
        ls_all[o:o + 9] = _lhs_rows(xs[:, k:k + 1])
        ls_all[o + 9:o + KS] = ones3
        rs_all[o:o + 9] = _rhs_rows(Ws[k:k + 1])
        rs_all[o + 9:o + KS] = bias_rows(bs[k])
    return lf, rf, lp, rp, ls_all, rs_all


def kernel(x, Wf, bf, Wp, bp, Ws, bs, _trace=False, _spmd_kwargs=None):
    from concourse.bass_utils import run_bass_kernel_spmd

    x = np.asarray(x, np.float32)
    lf, rf, lp, rp, ls_all, rs_all = _host_prep(
        x, np.asarray(Wf, np.float32), np.asarray(bf, np.float32),
        np.asarray(Wp, np.float32), np.asarray(bp, np.float32),
        np.asarray(Ws, np.float32), np.asarray(bs, np.float32))

    if "nc" not in _prog_cache:
        _prog_cache["nc"] = _build_program()
    nc = _prog_cache["nc"]

    in_maps = []
    for i in range(N_CORES):
        sl = slice(i * B_LOC, (i + 1) * B_LOC)
        m = {
            "lf": np.ascontiguousarray(lf[:, sl]),
            "lp": np.ascontiguousarray(lp[:, sl]),
            "rf": rf,
            "rp": rp,
        }
        for j, (a, b) in enumerate(S_TILES):
            rows = (b - a - 1) * S_STRIDE + KS
            o = S_STRIDE * a
            m[f"ls{j}"] = np.ascontiguousarray(ls_all[o:o + rows, sl])
            m[f"rs{j}"] = np.ascontiguousarray(rs_all[o:o + rows])
        in_maps.append(m)

    kwargs = dict(_spmd_kwargs or {})
    res = run_bass_kernel_spmd(nc, in_maps, core_ids=list(range(N_CORES)),
                               trace=_trace, **kwargs)
    out = np.concatenate(
        [np.asarray(r["out"]).astype(np.float32) for r in res.results], axis=0)
    if _trace:
        kernel.last_results = res
    return out.reshape(B, T, D)



# revision 18
# speedup vs baseline: 1.0282x; 1.0009x over previous
"""BoT tokenizer kernel for Trainium2 (Bass/Tile), 8-core data parallel.

All 25 output tokens are computed on the TensorEngine as bf16 matmuls with
an exact fp32 -> 3x bf16 mantissa split (8+8+8 = 24 bits):

    x = a0 + a1 + a2 (each bf16, split exact by construction)
    x*w = sum_{i,j} ai*wj   (each bf16 product is exact in fp32)

 - single-feature token k: K=12 matmul (9 cross products + 3 bias rows
   against a ones column)
 - fore token: 9 features -> K = 9*9+3 = 84
 - palm token: 7 features -> K = 7*9+3 = 66

bf16 matmuls stream 1 col/cycle (vs 4 for fp32), so the PE produces each
[128,512] token tile in ~215ns. PSUM->SBUF copies are split between
VectorE and ScalarE. The kernel is then purely output-DMA bound.

v2: the output is written to HBM as bf16 (down-converted during the
PSUM->SBUF copy) and up-cast to fp32 on the host. Compute stays exact;
the only error is the final bf16 rounding (~1e-3 normalized l2), well
inside the 2e-2 gate. This halves the dominant HBM write traffic:
each core writes 1024*25*512*2 = 26.2 MB. Singles inputs are stored
compact in HBM (no 32-row zero padding) and DMA'd into the padded
SBUF layout, cutting input DMA from 2.9 MB to 1.3 MB per core.
"""

import numpy as np

FORE_IDX = [0, 1, 2, 27, 28, 32, 33, 34, 38]
PALM_IDX = [4, 29, 30, 31, 35, 36, 37]
SINGLE_IDX = [3] + list(range(5, 27))

B = 8192
D = 512
T = 25
N_CORES = 8
B_LOC = B // N_CORES          # 1024 rows per core
CHUNK = 128
N_CHUNKS = B_LOC // CHUNK     # 8
ROW = T * D                   # 12800
NS = 23

# token id for single sensor k: k=0 -> token 1 (wrist), k>=1 -> token k+2
TOK_OF_SINGLE = [1] + list(range(3, 25))
# out-tile token groups for finer DMA pipelining
GROUPS = [(0, 6), (6, 12), (12, 19), (19, 25)]
KF = 9 * 9 + 3                # 84
KP = 7 * 9 + 3                # 66
KS = 12
# singles packed 3 per tile at 32-partition offsets (matmul base partition
# must be 0/32/64)
S_TILES = [(a, min(a + 3, NS)) for a in range(0, NS, 3)]
S_STRIDE = 32

_prog_cache = {}


def _k_of_tok(t):
    return 0 if t == 1 else t - 2


def _build_program():
    import concourse.bacc as bacc
    import concourse.mybir as mybir
    import concourse.tile as tile
    from concourse.bass import ts

    f32 = mybir.dt.float32
    bf16 = mybir.dt.bfloat16
    nc = bacc.Bacc("TRN2", target_bir_lowering=False, debug=False,
                   num_devices=N_CORES)

    lf_d = nc.dram_tensor("lf", [KF, B_LOC], bf16, kind="ExternalInput")
    lp_d = nc.dram_tensor("lp", [KP, B_LOC], bf16, kind="ExternalInput")
    rf_d = nc.dram_tensor("rf", [KF, D], bf16, kind="ExternalInput")
    rp_d = nc.dram_tensor("rp", [KP, D], bf16, kind="ExternalInput")
    ls_d = [nc.dram_tensor(f"ls{i}", [(b - a - 1) * S_STRIDE + KS, B_LOC],
                           bf16, kind="ExternalInput")
            for i, (a, b) in enumerate(S_TILES)]
    rs_d = [nc.dram_tensor(f"rs{i}", [(b - a - 1) * S_STRIDE + KS, D],
                           bf16, kind="ExternalInput")
            for i, (a, b) in enumerate(S_TILES)]
    out_d = nc.dram_tensor("out", [B_LOC, ROW], bf16, kind="ExternalOutput")

    with tile.TileContext(nc) as tc:
        with (
            tc.tile_pool(name="cst", bufs=1) as cst,
            tc.tile_pool(name="op", bufs=3) as op,
            tc.tile_pool(name="pp", bufs=4, space="PSUM") as pp,
        ):
            # Each dma_start costs ~700ns on its issuing sequencer, so use
            # one per tensor and keep vector/scalar free for PSUM copies:
            # gpsimd issues all lhsT loads, sync all rhs loads. The singles
            # are compact in HBM and land in strided 32-partition SBUF
            # slots via a rearranged AP in a single DMA each.
            lf_s = cst.tile([KF, B_LOC], bf16)
            nc.gpsimd.dma_start(out=lf_s[:], in_=lf_d[:])
            rf_s = cst.tile([KF, D], bf16)
            nc.sync.dma_start(out=rf_s[:], in_=rf_d[:])
            ls_s, rs_s = [], []
            for i, (a, b) in enumerate(S_TILES):
                rows = (b - a - 1) * S_STRIDE + KS
                lt = cst.tile([rows, B_LOC], bf16, name=f"ls{i}_s")
                nc.gpsimd.dma_start(out=lt[:], in_=ls_d[i][:])
                ls_s.append(lt)
                rt = cst.tile([rows, D], bf16, name=f"rs{i}_s")
                nc.sync.dma_start(out=rt[:], in_=rs_d[i][:])
                rs_s.append(rt)
                if i == 0:
                    # palm operands next: token 2 is needed right after
                    # fore (t0) and wrist (t1, sensor tile 0)
                    lp_s = cst.tile([KP, B_LOC], bf16)
                    nc.gpsimd.dma_start(out=lp_s[:], in_=lp_d[:])
                    rp_s = cst.tile([KP, D], bf16)
                    nc.sync.dma_start(out=rp_s[:], in_=rp_d[:])

            def operands(t, c):
                if t == 0:
                    return lf_s[:, ts(c, CHUNK)], rf_s[:]
                if t == 2:
                    return lp_s[:, ts(c, CHUNK)], rp_s[:]
                k = _k_of_tok(t)
                i = k // 3
                off = S_STRIDE * (k - S_TILES[i][0])
                return (ls_s[i][off:off + KS, ts(c, CHUNK)],
                        rs_s[i][off:off + KS, :])

            for c in range(N_CHUNKS):
                ncopy = 0
                if c == 0:
                    # first pair flushes alone: starts the output stream
                    # ~3-4us earlier while the pipeline is still filling
                    groups = [(0, 2), (2, 6)] + GROUPS[1:]
                elif c == N_CHUNKS - 1:
                    # fine-grained trailing groups shorten the final drain
                    groups = GROUPS[:3] + [(19, 22), (22, 25)]
                else:
                    groups = GROUPS
                for gi, (t0, t1) in enumerate(groups):
                    o_t = op.tile([CHUNK, (t1 - t0) * D], bf16,
                                  tag=f"out{c}_{gi}" if (c == 0 or
                                  c == N_CHUNKS - 1) else f"out{gi}",
                                  bufs=1 if (c == 0 or c == N_CHUNKS - 1)
                                  else 3)
                    # 2 matmuls -> one 2-bank PSUM tile. Measured per-token
                    # copy costs: scalar does pairs best (604ns/tok), vector
                    # does singles best (661ns/tok); alternating units gives
                    # scalar 13 tokens and vector 12 -> ~7.9us/chunk each.
                    t = t0
                    while t < t1:
                        te = min(t + 2, t1)
                        w = (te - t) * D
                        p_t = pp.tile([CHUNK, 2 * D], f32)
                        for tt in range(t, te):
                            lhsT, rhs = operands(tt, c)
                            nc.tensor.matmul(p_t[:, ts(tt - t, D)], lhsT,
                                             rhs, start=True, stop=True)
                        dst = o_t[:, (t - t0) * D:(te - t0) * D]
                        if ncopy % 2 == 0:
                            nc.scalar.copy(dst, p_t[:, 0:w])
                        else:
                            for j in range(te - t):
                                nc.vector.tensor_copy(
                                    o_t[:, (t - t0 + j) * D:(t - t0 + j + 1) * D],
                                    p_t[:, ts(j, D)])
                        ncopy += 1
                        t = te
                    dma_eng = nc.sync if gi % 2 == 0 else nc.gpsimd
                    dma_eng.dma_start(
                        out=out_d[ts(c, CHUNK), t0 * D:t1 * D], in_=o_t[:])

    nc.compile()
    return nc


def _split3(v):
    """Exact fp32 -> (bf16, bf16, bf16) mantissa split: v = s0+s1+s2."""
    import ml_dtypes
    bf = ml_dtypes.bfloat16
    v = np.asarray(v, np.float32)
    s0 = v.astype(bf)
    r1 = v - s0.astype(np.float32)
    s1 = r1.astype(bf)
    r2 = r1 - s1.astype(np.float32)
    s2 = r2.astype(bf)
    return s0, s1, s2


def _lhs_rows(xcols):
    """lhsT rows for a feature block: a0,a0,a0,a1,a1,a1,a2,a2,a2 per feat.

    xcols: [B, F] fp32 -> [9F, B] bf16"""
    import ml_dtypes
    Bn, F = xcols.shape
    s0, s1, s2 = _split3(xcols)          # each [B, F]
    out = np.empty((F, 9, Bn), dtype=ml_dtypes.bfloat16)
    for i, s in enumerate((s0, s1, s2)):
        out[:, 3 * i:3 * i + 3, :] = s.T[:, None, :]
    return out.reshape(9 * F, Bn)


def _rhs_rows(wcols):
    """rhs rows for a feature block: w0,w1,w2,w0,w1,w2,w0,w1,w2 per feat.

    wcols: [F, D] fp32 -> [9F, D] bf16"""
    import ml_dtypes
    F, Dn = wcols.shape
    s0, s1, s2 = _split3(wcols)
    out = np.empty((F, 3, 3, Dn), dtype=ml_dtypes.bfloat16)
    for j, s in enumerate((s0, s1, s2)):
        out[:, :, j, :] = s[:, None, :]
    return out.reshape(9 * F, Dn)


def _host_prep(x, Wf, bf_, Wp, bp, Ws, bs):
    import ml_dtypes
    bf16 = ml_dtypes.bfloat16

    ones3 = np.ones((3, B), dtype=bf16)

    def bias_rows(bias):
        b0, b1, b2 = _split3(bias)       # [D] each
        return np.stack([b0, b1, b2])    # [3, D]

    # fore: lhsT [84, B], rhs [84, D]
    lf = np.concatenate([_lhs_rows(x[:, FORE_IDX]), ones3])
    rf = np.concatenate([_rhs_rows(np.asarray(Wf.T)), bias_rows(bf_)])
    # palm: [66, *]
    lp = np.concatenate([_lhs_rows(x[:, PALM_IDX]), ones3])
    rp = np.concatenate([_rhs_rows(np.asarray(Wp.T)), bias_rows(bp)])

    # singles: per sensor a [12, *] block at 32-partition offsets (pad rows
    # zero) so each SBUF tile loads with one plain-AP DMA
    ls_all = np.zeros((NS * S_STRIDE, B), dtype=bf16)
    rs_all = np.zeros((NS * S_STRIDE, D), dtype=bf16)
    xs = x[:, SINGLE_IDX]                # [B, 23]
    for k in range(NS):
        o = S_STRIDE * k
        ls_all[o:o + 9] = _lhs_rows(xs[:, k:k + 1])
        ls_all[o + 9:o + KS] = ones3
        rs_all[o:o + 9] = _rhs_rows(Ws[k:k + 1])
        rs_all[o + 9:o + KS] = bias_rows(bs[k])
    return lf, rf, lp, rp, ls_all, rs_all


def kernel(x, Wf, bf, Wp, bp, Ws, bs, _trace=False, _spmd_kwargs=None):
    from concourse.bass_utils import run_bass_kernel_spmd

    x = np.asarray(x, np.float32)
    lf, rf, lp, rp, ls_all, rs_all = _host_prep(
        x, np.asarray(Wf, np.float32), np.asarray(bf, np.float32),
        np.asarray(Wp, np.float32), np.asarray(bp, np.float32),
        np.asarray(Ws, np.float32), np.asarray(bs, np.float32))

    if "nc" not in _prog_cache:
        _prog_cache["nc"] = _build_program()
    nc = _prog_cache["nc"]

    in_maps = []
    for i in range(N_CORES):
        sl = slice(i * B_LOC, (i + 1) * B_LOC)
        m = {
            "lf": np.ascontiguousarray(lf[:, sl]),
            "lp": np.ascontiguousarray(lp[:, sl]),
            "rf": rf,
            "rp": rp,
        }
        for j, (a, b) in enumerate(S_TILES):
            rows = (b - a - 1) * S_STRIDE + KS
            o = S_STRIDE * a
            m[f"ls{j}"] = np.ascontiguousarray(ls_all[o:o + rows, sl])
            m[f"rs{j}"] = np.ascontiguousarray(rs_all[o:o + rows])
        in_maps.append(m)

    kwargs = dict(_spmd_kwargs or {})
    res = run_bass_kernel_spmd(nc, in_maps, core_ids=list(range(N_CORES)),
                               trace=_trace, **kwargs)
    out = np.concatenate(
        [np.asarray(r["out"]).astype(np.float32) for r in res.results], axis=0)
    if _trace:
        kernel.last_results = res
    return out.reshape(B, T, D)



# revision 21
# speedup vs baseline: 1.0513x; 1.0225x over previous
"""BoT tokenizer kernel for Trainium2 (Bass/Tile), 8-core data parallel.

All 25 output tokens are computed on the TensorEngine as bf16 matmuls with
an exact fp32 -> 3x bf16 mantissa split (8+8+8 = 24 bits):

    x = a0 + a1 + a2 (each bf16, split exact by construction)
    x*w = sum_{i,j} ai*wj   (each bf16 product is exact in fp32)

 - single-feature token k: K=12 matmul (9 cross products + 3 bias rows
   against a ones column)
 - fore token: 9 features -> K = 9*9+3 = 84
 - palm token: 7 features -> K = 7*9+3 = 66

bf16 matmuls stream 1 col/cycle (vs 4 for fp32), so the PE produces each
[128,512] token tile in ~215ns. PSUM->SBUF copies are split between
VectorE and ScalarE. The kernel is then purely output-DMA bound.

v2: the output is written to HBM as bf16 (down-converted during the
PSUM->SBUF copy) and up-cast to fp32 on the host. Compute stays exact;
the only error is the final bf16 rounding (~1e-3 normalized l2), well
inside the 2e-2 gate. This halves the dominant HBM write traffic:
each core writes 1024*25*512*2 = 26.2 MB. Singles inputs are stored
compact in HBM (no 32-row zero padding) and DMA'd into the padded
SBUF layout, cutting input DMA from 2.9 MB to 1.3 MB per core.
"""

import numpy as np

FORE_IDX = [0, 1, 2, 27, 28, 32, 33, 34, 38]
PALM_IDX = [4, 29, 30, 31, 35, 36, 37]
SINGLE_IDX = [3] + list(range(5, 27))

B = 8192
D = 512
T = 25
N_CORES = 8
B_LOC = B // N_CORES          # 1024 rows per core
CHUNK = 128
N_CHUNKS = B_LOC // CHUNK     # 8
ROW = T * D                   # 12800
NS = 23

# token id for single sensor k: k=0 -> token 1 (wrist), k>=1 -> token k+2
TOK_OF_SINGLE = [1] + list(range(3, 25))
# out-tile token groups for finer DMA pipelining
GROUPS = [(0, 6), (6, 12), (12, 19), (19, 25)]
KF = 9 * 9 + 3                # 84
KP = 7 * 9 + 3                # 66
KS = 12
# singles packed 3 per tile at 32-partition offsets (matmul base partition
# must be 0/32/64)
S_TILES = [(a, min(a + 3, NS)) for a in range(0, NS, 3)]
S_STRIDE = 32

_prog_cache = {}


def _k_of_tok(t):
    return 0 if t == 1 else t - 2


def _build_program():
    import concourse.bacc as bacc
    import concourse.mybir as mybir
    import concourse.tile as tile
    from concourse.bass import ts

    f32 = mybir.dt.float32
    bf16 = mybir.dt.bfloat16
    nc = bacc.Bacc("TRN2", target_bir_lowering=False, debug=False,
                   num_devices=N_CORES)

    lf_d = nc.dram_tensor("lf", [KF, B_LOC], bf16, kind="ExternalInput")
    lp_d = nc.dram_tensor("lp", [KP, B_LOC], bf16, kind="ExternalInput")
    rf_d = nc.dram_tensor("rf", [KF, D], bf16, kind="ExternalInput")
    rp_d = nc.dram_tensor("rp", [KP, D], bf16, kind="ExternalInput")
    ls_d = [nc.dram_tensor(f"ls{i}", [(b - a - 1) * S_STRIDE + KS, B_LOC],
                           bf16, kind="ExternalInput")
            for i, (a, b) in enumerate(S_TILES)]
    rs_d = [nc.dram_tensor(f"rs{i}", [(b - a - 1) * S_STRIDE + KS, D],
                           bf16, kind="ExternalInput")
            for i, (a, b) in enumerate(S_TILES)]
    out_d = nc.dram_tensor("out", [B_LOC, ROW], bf16, kind="ExternalOutput")

    with tile.TileContext(nc) as tc:
        with (
            tc.tile_pool(name="cst", bufs=1) as cst,
            tc.tile_pool(name="op", bufs=3) as op,
            tc.tile_pool(name="pp", bufs=4, space="PSUM") as pp,
        ):
            # Each dma_start costs ~700ns on its issuing sequencer, so use
            # one per tensor and keep vector/scalar free for PSUM copies:
            # gpsimd issues all lhsT loads, sync all rhs loads. The singles
            # are compact in HBM and land in strided 32-partition SBUF
            # slots via a rearranged AP in a single DMA each.
            lf_s = cst.tile([KF, B_LOC], bf16)
            nc.gpsimd.dma_start(out=lf_s[:], in_=lf_d[:])
            rf_s = cst.tile([KF, D], bf16)
            nc.sync.dma_start(out=rf_s[:], in_=rf_d[:])
            lp_s = cst.tile([KP, B_LOC], bf16)
            nc.gpsimd.dma_start(out=lp_s[:], in_=lp_d[:])
            rp_s = cst.tile([KP, D], bf16)
            nc.sync.dma_start(out=rp_s[:], in_=rp_d[:])
            ls_s, rs_s = [], []
            for i, (a, b) in enumerate(S_TILES):
                rows = (b - a - 1) * S_STRIDE + KS
                lt = cst.tile([rows, B_LOC], bf16, name=f"ls{i}_s")
                nc.gpsimd.dma_start(out=lt[:], in_=ls_d[i][:])
                ls_s.append(lt)
                rt = cst.tile([rows, D], bf16, name=f"rs{i}_s")
                nc.sync.dma_start(out=rt[:], in_=rs_d[i][:])
                rs_s.append(rt)

            def operands(t, c):
                if t == 0:
                    return lf_s[:, ts(c, CHUNK)], rf_s[:]
                if t == 2:
                    return lp_s[:, ts(c, CHUNK)], rp_s[:]
                k = _k_of_tok(t)
                i = k // 3
                off = S_STRIDE * (k - S_TILES[i][0])
                return (ls_s[i][off:off + KS, ts(c, CHUNK)],
                        rs_s[i][off:off + KS, :])

            for c in range(N_CHUNKS):
                ncopy = 0
                if c == 0:
                    # emit fore and palm first (their operands arrive ~3us
                    # before the singles') and flush each alone: the output
                    # stream starts as early as possible
                    groups = [(0, 1), (2, 3), (1, 2), (3, 6)] + GROUPS[1:]
                elif c == N_CHUNKS - 1:
                    # fine-grained trailing groups shorten the final drain
                    groups = GROUPS[:3] + [(19, 22), (22, 25)]
                else:
                    groups = GROUPS
                for gi, (t0, t1) in enumerate(groups):
                    o_t = op.tile([CHUNK, (t1 - t0) * D], bf16,
                                  tag=f"out{c}_{gi}" if (c == 0 or
                                  c == N_CHUNKS - 1) else f"out{gi}",
                                  bufs=1 if (c == 0 or c == N_CHUNKS - 1)
                                  else 3)
                    # 2 matmuls -> one 2-bank PSUM tile. Measured per-token
                    # copy costs: scalar does pairs best (604ns/tok), vector
                    # does singles best (661ns/tok); alternating units gives
                    # scalar 13 tokens and vector 12 -> ~7.9us/chunk each.
                    t = t0
                    while t < t1:
                        te = min(t + 2, t1)
                        w = (te - t) * D
                        p_t = pp.tile([CHUNK, 2 * D], f32)
                        for tt in range(t, te):
                            lhsT, rhs = operands(tt, c)
                            nc.tensor.matmul(p_t[:, ts(tt - t, D)], lhsT,
                                             rhs, start=True, stop=True)
                        dst = o_t[:, (t - t0) * D:(te - t0) * D]
                        if ncopy % 2 == 0:
                            nc.scalar.copy(dst, p_t[:, 0:w])
                        else:
                            for j in range(te - t):
                                nc.vector.tensor_copy(
                                    o_t[:, (t - t0 + j) * D:(t - t0 + j + 1) * D],
                                    p_t[:, ts(j, D)])
                        ncopy += 1
                        t = te
                    dma_eng = nc.sync if gi % 2 == 0 else nc.gpsimd
                    dma_eng.dma_start(
                        out=out_d[ts(c, CHUNK), t0 * D:t1 * D], in_=o_t[:])

    nc.compile()
    return nc


def _split3(v):
    """Exact fp32 -> (bf16, bf16, bf16) mantissa split: v = s0+s1+s2."""
    import ml_dtypes
    bf = ml_dtypes.bfloat16
    v = np.asarray(v, np.float32)
    s0 = v.astype(bf)
    r1 = v - s0.astype(np.float32)
    s1 = r1.astype(bf)
    r2 = r1 - s1.astype(np.float32)
    s2 = r2.astype(bf)
    return s0, s1, s2


def _lhs_rows(xcols):
    """lhsT rows for a feature block: a0,a0,a0,a1,a1,a1,a2,a2,a2 per feat.

    xcols: [B, F] fp32 -> [9F, B] bf16"""
    import ml_dtypes
    Bn, F = xcols.shape
    s0, s1, s2 = _split3(xcols)          # each [B, F]
    out = np.empty((F, 9, Bn), dtype=ml_dtypes.bfloat16)
    for i, s in enumerate((s0, s1, s2)):
        out[:, 3 * i:3 * i + 3, :] = s.T[:, None, :]
    return out.reshape(9 * F, Bn)


def _rhs_rows(wcols):
    """rhs rows for a feature block: w0,w1,w2,w0,w1,w2,w0,w1,w2 per feat.

    wcols: [F, D] fp32 -> [9F, D] bf16"""
    import ml_dtypes
    F, Dn = wcols.shape
    s0, s1, s2 = _split3(wcols)
    out = np.empty((F, 3, 3, Dn), dtype=ml_dtypes.bfloat16)
    for j, s in enumerate((s0, s1, s2)):
        out[:, :, j, :] = s[:, None, :]
    return out.reshape(9 * F, Dn)


def _host_prep(x, Wf, bf_, Wp, bp, Ws, bs):
    import ml_dtypes
    bf16 = ml_dtypes.bfloat16

    ones3 = np.ones((3, B), dtype=bf16)

    def bias_rows(bias):
        b0, b1, b2 = _split3(bias)       # [D] each
        return np.stack([b0, b1, b2])    # [3, D]

    # fore: lhsT [84, B], rhs [84, D]
    lf = np.concatenate([_lhs_rows(x[:, FORE_IDX]), ones3])
    rf = np.concatenate([_rhs_rows(np.asarray(Wf.T)), bias_rows(bf_)])
    # palm: [66, *]
    lp = np.concatenate([_lhs_rows(x[:, PALM_IDX]), ones3])
    rp = np.concatenate([_rhs_rows(np.asarray(Wp.T)), bias_rows(bp)])

    # singles: per sensor a [12, *] block at 32-partition offsets (pad rows
    # zero) so each SBUF tile loads with one plain-AP DMA
    ls_all = np.zeros((NS * S_STRIDE, B), dtype=bf16)
    rs_all = np.zeros((NS * S_STRIDE, D), dtype=bf16)
    xs = x[:, SINGLE_IDX]                # [B, 23]
    for k in range(NS):
        o = S_STRIDE * k
        ls_all[o:o + 9] = _lhs_rows(xs[:, k:k + 1])
        ls_all[o + 9:o + KS] = ones3
        rs_all[o:o + 9] = _rhs_rows(Ws[k:k + 1])
        rs_all[o + 9:o + KS] = bias_rows(bs[k])
    return lf, rf, lp, rp, ls_all, rs_all


def kernel(x, Wf, bf, Wp, bp, Ws, bs, _trace=False, _spmd_kwargs=None):
    from concourse.bass_utils import run_bass_kernel_spmd

    x = np.asarray(x, np.float32)
    lf, rf, lp, rp, ls_all, rs_all = _host_prep(
        x, np.asarray(Wf, np.float32), np.asarray(bf, np.float32),
        np.asarray(Wp, np.float32), np.asarray(bp, np.float32),
        np.asarray(Ws, np.float32), np.asarray(bs, np.float32))

    if "nc" not in _prog_cache:
        _prog_cache["nc"] = _build_program()
    nc = _prog_cache["nc"]

    in_maps = []
    for i in range(N_CORES):
        sl = slice(i * B_LOC, (i + 1) * B_LOC)
        m = {
            "lf": np.ascontiguousarray(lf[:, sl]),
            "lp": np.ascontiguousarray(lp[:, sl]),
            "rf": rf,
            "rp": rp,
        }
        for j, (a, b) in enumerate(S_TILES):
            rows = (b - a - 1) * S_STRIDE + KS
            o = S_STRIDE * a
            m[f"ls{j}"] = np.ascontiguousarray(ls_all[o:o + rows, sl])
            m[f"rs{j}"] = np.ascontiguousarray(rs_all[o:o + rows])
        in_maps.append(m)

    kwargs = dict(_spmd_kwargs or {})
    res = run_bass_kernel_spmd(nc, in_maps, core_ids=list(range(N_CORES)),
                               trace=_trace, **kwargs)
    out = np.concatenate(
        [np.asarray(r["out"]).astype(np.float32) for r in res.results], axis=0)
    if _trace:
        kernel.last_results = res
    return out.reshape(B, T, D)

